# revision 9
# baseline (speedup 1.0000x reference)
"""Causal self-attention (B=2, T=2048, C=1024, H=16, D=64) on 8 TRN2 NeuronCores.

Sharding: core = 4*b + g  (b in {0,1} batch, g in {0..3} head-group of 4 heads).
Each core computes, for its batch element and its 4 heads:
    qkv slice -> causal attention -> partial output projection
and returns a [2048, 1024] bf16 partial product of out = y @ w_proj. The host
sums the 4 head-group partials per batch (in f32) and adds the bias terms that
commute out exactly:
    out += b_qkv[v-part] @ w_proj + b_proj      (softmax rows sum to 1)
b_q / b_k are applied on-device (per-partition bias on the Q^T/K^T copy).

All matmuls run in bf16 (full PE rate; fp32 PSUM accumulation). x arrives
pre-transposed/pre-cast to bf16 from the host, so there is no on-device
transpose.

Scheduling: the PE HAM clock gate runs the PE at 1.2 GHz unless it is
continuously busy. The attention inner loop alone is ACT-exp-paced, which
starves the PE. So QKV work for chunk c+1 and the output projection for chunk
c-1 are statically interleaved between the S-tile units of chunk c as PE
"filler": causality means attention chunk c only needs Q/K/V of chunks <= c.
Adjacent full S-tiles share a 2-bank PSUM tile so their exp runs as ONE ACT
instruction (halves ACT per-instruction overhead). The softmax 1/den broadcast
runs on the otherwise-idle GPSIMD engine (partition_broadcast), not the PE.
Input DMAs are spread across engine queues so they load in parallel.

Per-head slot structure (in PE program order):
    [1/den broadcast + yT write of the PREVIOUS head]
    S-units (pairs of full tiles + packed diagonal tiles), filler between units
    remaining filler
    AV-burst (row 64 of yp = softmax denominator via the ones-column of V')
    1/den chain on DVE (copy den row out of PSUM -> approx reciprocal -> bf16)
      [reciprocal_approx_fast misreads PSUM on HW; the SBUF staging copy is
       required, not just faster]
"""

import numpy as np
import ml_dtypes
from contextlib import ExitStack

import concourse.bass as bass
import concourse.mybir as mybir
import concourse.tile as tile
from concourse import bacc
from concourse.bass_utils import run_bass_kernel_spmd

F32 = mybir.dt.float32
BF16 = mybir.dt.bfloat16
Exp = mybir.ActivationFunctionType.Exp
ADD = mybir.AluOpType.add
MUL = mybir.AluOpType.mult

C = 1024
NKC = C // 128  # 8 contraction tiles over channels
HL = 4          # local heads per core
D = 64

BF = ml_dtypes.bfloat16


def build_nc(T: int = 2048, enable_asserts: bool = False) -> bass.Bass:
    TT = T // 128   # T tiles
    TC = T // 512   # T chunks
    assert TT == 4 * TC and TC == 4

    nc = bacc.Bacc(
        "TRN2",
        target_bir_lowering=False,
        debug=False,
        enable_asserts=enable_asserts,
        num_devices=8,
    )
    xt_d = nc.dram_tensor("xt", [C, T], BF16, kind="ExternalInput").ap()
    wqkv_d = nc.dram_tensor("wqkv", [C, 768], BF16, kind="ExternalInput").ap()
    bqkv_d = nc.dram_tensor("bqkv", [512], F32, kind="ExternalInput").ap()
    wproj_d = nc.dram_tensor("wproj", [256, C], BF16, kind="ExternalInput").ap()
    out_d = nc.dram_tensor("out", [T, C], BF16, kind="ExternalOutput").ap()

    with tile.TileContext(nc) as tc, ExitStack() as ctx:
        const = ctx.enter_context(tc.tile_pool(name="const", bufs=1))
        main = ctx.enter_context(tc.tile_pool(name="main", bufs=1))
        pt_pool = ctx.enter_context(tc.tile_pool(name="pt", bufs=10))
        small = ctx.enter_context(tc.tile_pool(name="small", bufs=2))
        ych_pool = ctx.enter_context(tc.tile_pool(name="ych", bufs=2))
        out_pool = ctx.enter_context(tc.tile_pool(name="osb", bufs=2))

        ps_misc = ctx.enter_context(tc.tile_pool(name="ps_misc", bufs=2, space="PSUM"))
        ps_s = ctx.enter_context(tc.tile_pool(name="ps_s", bufs=2, space="PSUM"))
        ps_y = ctx.enter_context(tc.tile_pool(name="ps_y", bufs=2, space="PSUM"))

        # inputs: spread DMAs across engine queues so they load in parallel
        wqkv_sb = main.tile([128, NKC, 768], BF16)
        nc.sync.dma_start(wqkv_sb, wqkv_d.rearrange("(ko p) n -> p ko n", p=128))
        bqk = const.tile([128, 4], F32)
        nc.sync.dma_start(bqk, bqkv_d.rearrange("(m p) -> p m", p=128))

        # x^T resident in SBUF: xt[p, kc, t] = x[t, 128kc+p]
        xt = main.tile([128, NKC, T], BF16)
        xtr = xt_d.rearrange("(kc p) t -> p kc t", p=128)
        dma_engines = [nc.scalar, nc.gpsimd, nc.scalar, nc.gpsimd]
        for th in range(2):
            for kh in range(2):
                dma_engines[2 * th + kh].dma_start(
                    xt[:, 4 * kh : 4 * (kh + 1), 1024 * th : 1024 * (th + 1)],
                    xtr[:, 4 * kh : 4 * (kh + 1), 1024 * th : 1024 * (th + 1)],
                )
        wproj_sb = main.tile([128, 2, C], BF16)
        nc.sync.dma_start(wproj_sb, wproj_d.rearrange("(ko p) n -> p ko n", p=128))

        ones64 = const.tile([1, 64], BF16)
        nc.vector.memset(ones64, 1.0)

        # qkT[p, m, t] = (x @ w_qk + b_qk)^T at channel u=128m+p (u<256: Q, else K)
        qkT = main.tile([128, 4, T], BF16)
        # vsb[p, tt, 65h+d] = V[128tt+p, 64h+d]; column 65h+64 = 1.0
        vsb = main.tile([128, TT, HL * 65], BF16)
        v4 = vsb.rearrange("p t (h e) -> p t h e", e=65)
        nc.vector.memset(v4[:, :, :, 64:65], 1.0)

        # ---- schedulable PE work units -------------------------------------
        def qk_band(m, t):
            ps = ps_misc.tile([128, 512], F32, tag="misc", name=f"qkps_{m}_{t}")
            for kc in range(NKC):
                nc.tensor.matmul(
                    ps,
                    lhsT=wqkv_sb[:, kc, 128 * m : 128 * (m + 1)],
                    rhs=xt[:, kc, 512 * t : 512 * (t + 1)],
                    start=(kc == 0),
                    stop=(kc == NKC - 1),
                )
            nc.vector.tensor_tensor(
                qkT[:, m, 512 * t : 512 * (t + 1)],
                ps,
                bqk[:, m : m + 1].to_broadcast([128, 512]),
                ADD,
            )

        def v_tile(i):
            ps = ps_misc.tile([128, 512], F32, tag="misc", name=f"vps_{i}")
            for kc in range(NKC):
                nc.tensor.matmul(
                    ps[:, 0:256],
                    lhsT=xt[:, kc, 128 * i : 128 * (i + 1)],
                    rhs=wqkv_sb[:, kc, 512:768],
                    start=(kc == 0),
                    stop=(kc == NKC - 1),
                )
            nc.vector.tensor_copy(
                v4[:, i, :, 0:64], ps[:, 0:256].rearrange("p (h e) -> p h e", e=64)
            )

        yTs = {}

        def proj_unit(c, tl):
            yT = yTs[c]
            osb = out_pool.tile([128, C], BF16, tag="osb", name=f"osb_{c}_{tl}")
            for nn in range(2):
                pp = ps_misc.tile([128, 512], F32, tag="misc", name=f"pp_{c}_{tl}_{nn}")
                for j in range(2):
                    nc.tensor.matmul(
                        pp,
                        lhsT=yT[:, j, 128 * tl : 128 * (tl + 1)],
                        rhs=wproj_sb[:, j, 512 * nn : 512 * (nn + 1)],
                        start=(j == 0),
                        stop=(j == 1),
                    )
                nc.vector.tensor_copy(osb[:, 512 * nn : 512 * (nn + 1)], pp)
                r0 = 512 * c + 128 * tl
                nc.sync.dma_start(
                    out_d[r0 : r0 + 128, 512 * nn : 512 * (nn + 1)],
                    osb[:, 512 * nn : 512 * (nn + 1)],
                )

        def bc_mul(st):
            """Broadcast 1/den across the 64 head dims (GPSIMD partition
            broadcast) and write the normalized yT band."""
            yp, rdenb, yT, po, mq, tag = st
            bcs = small.tile([64, 512], BF16, tag="bc", name=f"bc_{tag}")
            nc.gpsimd.partition_broadcast(bcs, rdenb)
            nc.vector.tensor_tensor(yT[po : po + 64, mq, :], yp[0:64, 0:512], bcs, MUL)

        # static filler assignment: fillers[c][h] emitted inside slot (c,h).
        # QKV(c) must land before chunk c's S/AV reads it; proj(c) units only
        # after bc_mul(c, 3) (slot (c+1, 0)) to keep the in-order PE queue
        # deadlock-free.
        fillers = {(c, h): [] for c in range(TC) for h in range(HL)}
        fillers[0, 0] = [lambda: qk_band(0, 1), lambda: qk_band(1, 1)]
        fillers[0, 1] = [lambda: qk_band(2, 1), lambda: qk_band(3, 1)]
        fillers[0, 2] = [lambda: v_tile(4), lambda: v_tile(5)]
        fillers[0, 3] = [lambda: v_tile(6), lambda: v_tile(7)]
        fillers[1, 0] = [lambda: qk_band(0, 2), lambda: qk_band(1, 2)]
        fillers[1, 1] = [lambda: qk_band(2, 2), lambda: qk_band(3, 2), lambda: proj_unit(0, 0)]
        fillers[1, 2] = [lambda: v_tile(8), lambda: v_tile(9), lambda: proj_unit(0, 1)]
        fillers[1, 3] = [lambda: v_tile(10), lambda: v_tile(11), lambda: proj_unit(0, 2)]
        fillers[2, 0] = [lambda: proj_unit(0, 3), lambda: qk_band(0, 3), lambda: qk_band(2, 3)]
        fillers[2, 1] = [lambda: proj_unit(1, 0), lambda: proj_unit(1, 1), lambda: proj_unit(1, 2)]
        fillers[2, 2] = [lambda: proj_unit(1, 3), lambda: v_tile(12)]
        fillers[2, 3] = [lambda: v_tile(13), lambda: v_tile(14)]
        fillers[3, 0] = [lambda: v_tile(15), lambda: qk_band(1, 3)]
        fillers[3, 1] = [lambda: qk_band(3, 3), lambda: proj_unit(2, 0)]
        fillers[3, 2] = [lambda: proj_unit(2, 1), lambda: proj_unit(2, 2)]
        fillers[3, 3] = [lambda: proj_unit(2, 3)]

        # ---- QKV for chunk 0, then the interleaved attention schedule ------
        for m in range(4):
            qk_band(m, 0)
        for i in range(4):
            v_tile(i)

        prev = None  # pending bc_mul state from the previous head slot
        for c in range(TC):
            yTs[c] = ych_pool.tile([128, 2, 512], BF16, tag="yT", name=f"yT_{c}")
            for h in range(HL):
                po = 64 * (h % 2)
                mq = h // 2
                mk = 2 + h // 2
                if prev is not None:
                    bc_mul(prev)
                    prev = None
                fq = list(fillers[c, h])
                # S-units: pairs of full tiles, then diagonal tiles packed two
                # per 2-bank PSUM tile (separate exps; affine_select masks)
                units = [("pair", [a, a + 1]) for a in range(0, 4 * c, 2)]
                units += [("diag", [4 * c, 4 * c + 1]), ("diag", [4 * c + 2, 4 * c + 3])]
                pts = []
                for ui, (kind, tks) in enumerate(units):
                    sp = ps_s.tile([128, 2, 512], F32, tag="sp", name=f"sp_{c}_{h}_{ui}")
                    pt = pt_pool.tile([128, 2, 512], BF16, tag="pt", name=f"pt_{c}_{h}_{ui}")
                    offs = []
                    for sl, tk in enumerate(tks):
                        jd = tk - 4 * c  # >=0 on diagonal tiles
                        off = 128 * jd if jd > 0 else 0
                        offs.append(off)
                        nc.tensor.matmul(
                            sp[:, sl, off:512],
                            lhsT=qkT[po : po + 64, mk, 128 * tk : 128 * (tk + 1)],
                            rhs=qkT[po : po + 64, mq, 512 * c + off : 512 * (c + 1)],
                            start=True,
                            stop=True,
                        )
                    if kind == "pair":
                        nc.scalar.activation(pt[:, :, :], sp[:, :, :], Exp, scale=0.125)
                    else:
                        for sl, tk in enumerate(tks):
                            jd = tk - 4 * c
                            off = offs[sl]
                            nc.scalar.activation(
                                pt[:, sl, off:512], sp[:, sl, off:512], Exp, scale=0.125
                            )
                            # zero k>q inside the [128,128] diagonal block
                            nc.gpsimd.affine_select(
                                out=pt[:, sl, 128 * jd : 128 * (jd + 1)],
                                in_=pt[:, sl, 128 * jd : 128 * (jd + 1)],
                                compare_op=mybir.AluOpType.is_ge,
                                fill=0.0,
                                base=0,
                                pattern=[[1, 128]],
                                channel_multiplier=-1,
                            )
                    for sl, tk in enumerate(tks):
                        pts.append((tk, pt, sl, offs[sl]))
                    if ui % 2 == 0 and fq:
                        fq.pop(0)()
                for f in fq:
                    f()

                yp = ps_y.tile([128, 512], F32, tag="yp", name=f"yp_{c}_{h}")
                for n, (tk, pt, sl, off) in enumerate(pts):
                    nc.tensor.matmul(
                        yp[0:65, off:512],
                        lhsT=vsb[:, tk, 65 * h : 65 * (h + 1)],
                        rhs=pt[:, sl, off:512],
                        start=(n == 0),
                        stop=(n == len(pts) - 1),
                    )

                # 1/den chain (DVE): reciprocal_approx_fast misreads PSUM on
                # HW, so the denominator row must be staged to SBUF first
                denrow = small.tile([1, 512], F32, tag="denrow", name=f"denrow_{c}_{h}")
                nc.vector.tensor_copy(denrow, yp[64:65, 0:512])
                rden = small.tile([1, 512], F32, tag="rden", name=f"rden_{c}_{h}")
                nc.vector.reciprocal_approx_fast(rden, denrow)
                rdenb = small.tile([1, 512], BF16, tag="rdenb", name=f"rdenb_{c}_{h}")
                nc.vector.tensor_copy(rdenb, rden)
                prev = (yp, rdenb, yTs[c], po, mq, f"{c}_{h}")

        bc_mul(prev)
        for tl in range(4):
            proj_unit(3, tl)

    nc.finalize()  # runs Bacc register allocation; walrus rejects unfinalized BIR
    return nc


_NC_CACHE = {}


def _get_nc(T: int = 2048) -> bass.Bass:
    if T not in _NC_CACHE:
        _NC_CACHE[T] = build_nc(T)
    return _NC_CACHE[T]


def make_in_maps(x, w_qkv, b_qkv, w_proj):
    """Shard full inputs into 8 per-core input maps (core = 4*b + g)."""
    x = np.asarray(x, dtype=np.float32)
    w_qkv = np.asarray(w_qkv, dtype=np.float32)
    b_qkv = np.asarray(b_qkv, dtype=np.float32)
    w_proj = np.asarray(w_proj, dtype=np.float32)
    xts = [np.ascontiguousarray(x[b].T.astype(BF)) for b in range(x.shape[0])]
    in_maps = []
    for core in range(8):
        b, g = core // 4, core % 4
        gs = slice(256 * g, 256 * (g + 1))
        wl = np.ascontiguousarray(
            np.concatenate(
                [w_qkv[:, gs], w_qkv[:, 1024:2048][:, gs], w_qkv[:, 2048:3072][:, gs]],
                axis=1,
            ).astype(BF)
        )
        bl = np.ascontiguousarray(
            np.concatenate([b_qkv[0:1024][gs], b_qkv[1024:2048][gs]])
        )
        wp = np.ascontiguousarray(w_proj[gs, :].astype(BF))
        in_maps.append({"xt": xts[b], "wqkv": wl, "bqkv": bl, "wproj": wp})
    return in_maps


def combine_outputs(results, b_qkv, w_proj, b_proj):
    """Unshard: sum the 4 head-group partials per batch, add commuted biases."""
    b_qkv = np.asarray(b_qkv, dtype=np.float32)
    w_proj = np.asarray(w_proj, dtype=np.float32)
    b_proj = np.asarray(b_proj, dtype=np.float32)
    outs = [r["out"].astype(np.float32) for r in results]
    out = np.stack(
        [
            outs[0] + outs[1] + outs[2] + outs[3],
            outs[4] + outs[5] + outs[6] + outs[7],
        ]
    ).astype(np.float32)
    out += (b_qkv[2048:3072] @ w_proj + b_proj)[None, None, :]
    return out


def kernel(x, w_qkv, b_qkv, w_proj, b_proj):
    in_maps = make_in_maps(x, w_qkv, b_qkv, w_proj)
    res = run_bass_kernel_spmd(_get_nc(2048), in_maps, core_ids=list(range(8)))
    return combine_outputs(res.results, b_qkv, w_proj, b_proj)


def run_traced(x, w_qkv, b_qkv, w_proj, b_proj, trace_cores=None):
    """Like kernel(), but returns (output, BassKernelResults) with an NTFF trace."""
    in_maps = make_in_maps(x, w_qkv, b_qkv, w_proj)
    res = run_bass_kernel_spmd(
        _get_nc(2048),
        in_maps,
        core_ids=list(range(8)),
        trace=True,
        trace_cores=trace_cores if trace_cores is not None else [0],
    )
    return combine_outputs(res.results, b_qkv, w_proj, b_proj), res
